# revision 8
# baseline (speedup 1.0000x reference)
"""Reverse-time forget-mult recurrence on 8 Trainium2 NeuronCores.

h_t = f_t*x_t + (1-f_t)*h_{t+1}, h_{T+1}=0, over [T=2048, B=16, D=1024].

Strategy: shard D across the 8 cores (128 channels each) — the recurrence is
elementwise over (B, D), sequential only in T, so no cross-core communication.
On the host, each core's shard is laid out partition-major as [D_shard=128,
B=16, T] with the T axis reversed, so each (d, b) lane's full time series is
contiguous and the device scans forward. Per 2-block step the device does one
contiguous 2 MB DMA per tensor (16 KB per-partition lines), computes
a = 1-f on the Scalar engine and g = f*x on the Vector engine, and runs the
whole recurrence for 128 lanes x 2048 steps in a single hardware
tensor_tensor_scan instruction (initial state 0) on Vector. Loads issue on
the Sync HWDGE ring, stores on the Scalar ring, so writes don't
head-of-line-block reads. The kernel is memory-bound: 48 MB of HBM traffic
per core.
"""

import numpy as np

T, B, D = 2048, 16, 1024
NCORES = 8
DS = D // NCORES          # 128 channels per core -> the SBUF partition dim
NBLK = B                  # 16 blocks of [128, T] per core
RB = 2                    # row-blocks per DMA (2 MB transfers)
PB = 128

_cached = {}


def _build():
    import concourse.bacc as bacc
    import concourse.mybir as mybir
    import concourse.tile as tile

    f32 = mybir.dt.float32
    nc = bacc.Bacc("TRN2", target_bir_lowering=False, debug=False, num_devices=NCORES)
    f_in = nc.dram_tensor("f_in", [PB, NBLK, T], f32, kind="ExternalInput").ap()
    x_in = nc.dram_tensor("x_in", [PB, NBLK, T], f32, kind="ExternalInput").ap()
    h_out = nc.dram_tensor("h_out", [PB, NBLK, T], f32, kind="ExternalOutput").ap()

    nsteps = NBLK // RB
    H = T // 2
    with tile.TileContext(nc) as tc:
        with (
            tc.tile_pool(name="io", bufs=3) as io_pool,
            tc.tile_pool(name="hp", bufs=3) as h_pool,
            tc.tile_pool(name="tmp", bufs=2) as tmp_pool,
        ):
            for r in range(nsteps - 1):
                bsl = slice(RB * r, RB * (r + 1))
                f_t = io_pool.tile([PB, RB, T], f32, tag="f")
                nc.sync.dma_start(out=f_t[:], in_=f_in[:, bsl, :])
                x_t = io_pool.tile([PB, RB, T], f32, tag="x")
                nc.sync.dma_start(out=x_t[:], in_=x_in[:, bsl, :])
                h_t = h_pool.tile([PB, RB, T], f32, tag="h")
                for j in range(RB):
                    a_t = tmp_pool.tile([PB, T], f32, tag="a")
                    nc.scalar.activation(
                        a_t[:], f_t[:, j, :],
                        mybir.ActivationFunctionType.Copy, bias=1.0, scale=-1.0,
                    )
                    g_t = tmp_pool.tile([PB, T], f32, tag="g")
                    nc.vector.tensor_mul(g_t[:], f_t[:, j, :], x_t[:, j, :])
                    nc.vector.tensor_tensor_scan(
                        h_t[:, j, :], a_t[:], g_t[:], 0.0,
                        mybir.AluOpType.mult, mybir.AluOpType.add,
                    )
                nc.scalar.dma_start(out=h_out[:, bsl, :], in_=h_t[:])

            # Final step: half-T granularity to shorten the pipeline drain —
            # chained half-scans, half-sized loads and stores.
            r = nsteps - 1
            bsl = slice(RB * r, RB * (r + 1))
            f_h = [io_pool.tile([PB, RB, H], f32, tag="f", name=f"fh{i}") for i in range(2)]
            x_h = [io_pool.tile([PB, RB, H], f32, tag="x", name=f"xh{i}") for i in range(2)]
            for half in range(2):
                csl = slice(H * half, H * (half + 1))
                nc.sync.dma_start(out=f_h[half][:], in_=f_in[:, bsl, csl])
                nc.sync.dma_start(out=x_h[half][:], in_=x_in[:, bsl, csl])
            h_f = [h_pool.tile([PB, T], f32, tag="h", name=f"hf{i}") for i in range(RB)]
            for half in range(2):
                csl = slice(H * half, H * (half + 1))
                for j in range(RB):
                    a_t = tmp_pool.tile([PB, H], f32, tag="a")
                    nc.scalar.activation(
                        a_t[:], f_h[half][:, j, :],
                        mybir.ActivationFunctionType.Copy, bias=1.0, scale=-1.0,
                    )
                    g_t = tmp_pool.tile([PB, H], f32, tag="g")
                    nc.vector.tensor_mul(g_t[:], f_h[half][:, j, :], x_h[half][:, j, :])
                    init = 0.0 if half == 0 else h_f[j][:, H - 1 : H]
                    nc.vector.tensor_tensor_scan(
                        h_f[j][:, csl], a_t[:], g_t[:], init,
                        mybir.AluOpType.mult, mybir.AluOpType.add,
                    )
                    nc.scalar.dma_start(
                        out=h_out[:, RB * r + j, csl], in_=h_f[j][:, csl]
                    )
    nc.compile()
    return nc


def _get_nc():
    if "nc" not in _cached:
        _cached["nc"] = _build()
    return _cached["nc"]


def _shard(arr):
    """[T, B, D] -> per-core [DS, B, T] (partition-major) with T reversed."""
    v = arr[::-1].transpose(2, 1, 0)  # [D, B, T] strided view, T reversed
    return [
        np.ascontiguousarray(v[DS * c : DS * (c + 1)]) for c in range(NCORES)
    ]


def _run(f, x, trace=False):
    from concourse.bass_utils import run_bass_kernel_spmd

    f = np.asarray(f, dtype=np.float32)
    x = np.asarray(x, dtype=np.float32)
    assert f.shape == (T, B, D) and x.shape == (T, B, D)

    nc = _get_nc()
    f_shards = _shard(f)
    x_shards = _shard(x)
    in_maps = [{"f_in": f_shards[c], "x_in": x_shards[c]} for c in range(NCORES)]
    res = run_bass_kernel_spmd(nc, in_maps, core_ids=list(range(NCORES)), trace=trace)

    out = np.empty((T, B, D), dtype=np.float32)
    for c in range(NCORES):
        # h_c[d, b, t_rev] -> out[t, b, DS*c + d]
        out[:, :, DS * c : DS * (c + 1)] = res.results[c]["h_out"][:, :, ::-1].transpose(2, 1, 0)
    return out.reshape(T * B, D), res


def kernel(f, x):
    return _run(f, x, trace=False)[0]


# revision 9
# speedup vs baseline: 1.1400x; 1.1400x over previous
"""Reverse-time forget-mult recurrence on 8 Trainium2 NeuronCores.

h_t = f_t*x_t + (1-f_t)*h_{t+1}, h_{T+1}=0, over [T=2048, B=16, D=1024].

Strategy: shard D across the 8 cores (128 channels each) — the recurrence is
elementwise over (B, D), sequential only in T, so no cross-core communication.
On the host, each core's shard is laid out partition-major as [D_shard=128,
B=16, T] with the T axis reversed, so each (d, b) lane's full time series is
contiguous and the device scans forward. Per 2-block step the device does one
contiguous 2 MB DMA per tensor (16 KB per-partition lines), computes
a = 1-f on the Scalar engine and g = f*x on the Vector engine, and runs the
whole recurrence for 128 lanes x 2048 steps in a single hardware
tensor_tensor_scan instruction (initial state 0) on Vector. Loads issue on
the Sync HWDGE ring, stores on the Scalar ring, so writes don't
head-of-line-block reads. The very last block is scanned/stored in chained
quarter-T chunks to shorten the pipeline drain. The kernel is memory-bound:
48 MB of HBM traffic per core.
"""

import numpy as np

T, B, D = 2048, 16, 1024
NCORES = 8
DS = D // NCORES          # 128 channels per core -> the SBUF partition dim
NBLK = B                  # 16 blocks of [128, T] per core
RB = 2                    # row-blocks per DMA (2 MB transfers)
PB = 128

_cached = {}


def _build():
    import concourse.bacc as bacc
    import concourse.mybir as mybir
    import concourse.tile as tile

    f32 = mybir.dt.float32
    nc = bacc.Bacc("TRN2", target_bir_lowering=False, debug=False, num_devices=NCORES)
    f_in = nc.dram_tensor("f_in", [PB, NBLK, T], f32, kind="ExternalInput").ap()
    x_in = nc.dram_tensor("x_in", [PB, NBLK, T], f32, kind="ExternalInput").ap()
    h_out = nc.dram_tensor("h_out", [PB, NBLK, T], f32, kind="ExternalOutput").ap()

    nsteps = NBLK // RB
    Q = T // 4
    with tile.TileContext(nc) as tc:
        with (
            tc.tile_pool(name="io", bufs=3) as io_pool,
            tc.tile_pool(name="hp", bufs=4) as h_pool,
            tc.tile_pool(name="tmp", bufs=3) as tmp_pool,
        ):
            for r in range(nsteps):
                bsl = slice(RB * r, RB * (r + 1))
                f_t = io_pool.tile([PB, RB, T], f32, tag="f")
                nc.sync.dma_start(out=f_t[:], in_=f_in[:, bsl, :])
                x_t = io_pool.tile([PB, RB, T], f32, tag="x")
                nc.sync.dma_start(out=x_t[:], in_=x_in[:, bsl, :])
                for j in range(RB):
                    blk = RB * r + j
                    a_t = tmp_pool.tile([PB, T], f32, tag="a")
                    nc.scalar.activation(
                        a_t[:], f_t[:, j, :],
                        mybir.ActivationFunctionType.Copy, bias=1.0, scale=-1.0,
                    )
                    g_t = tmp_pool.tile([PB, T], f32, tag="g")
                    nc.vector.tensor_mul(g_t[:], f_t[:, j, :], x_t[:, j, :])
                    h_t = h_pool.tile([PB, T], f32, tag="h")
                    if blk < NBLK - 1:
                        nc.vector.tensor_tensor_scan(
                            h_t[:], a_t[:], g_t[:], 0.0,
                            mybir.AluOpType.mult, mybir.AluOpType.add,
                        )
                        nc.scalar.dma_start(out=h_out[:, blk, :], in_=h_t[:])
                    else:
                        # last block: chained quarter-scans + quarter-stores
                        # to shorten the pipeline drain
                        for q in range(4):
                            qsl = slice(Q * q, Q * (q + 1))
                            init = 0.0 if q == 0 else h_t[:, Q * q - 1 : Q * q]
                            nc.vector.tensor_tensor_scan(
                                h_t[:, qsl], a_t[:, qsl], g_t[:, qsl], init,
                                mybir.AluOpType.mult, mybir.AluOpType.add,
                            )
                            nc.scalar.dma_start(
                                out=h_out[:, blk, qsl], in_=h_t[:, qsl]
                            )
    nc.compile()
    return nc


def _get_nc():
    if "nc" not in _cached:
        _cached["nc"] = _build()
    return _cached["nc"]


def _shard(arr):
    """[T, B, D] -> per-core [DS, B, T] (partition-major) with T reversed."""
    v = arr[::-1].transpose(2, 1, 0)  # [D, B, T] strided view, T reversed
    return [
        np.ascontiguousarray(v[DS * c : DS * (c + 1)]) for c in range(NCORES)
    ]


def _run(f, x, trace=False):
    from concourse.bass_utils import run_bass_kernel_spmd

    f = np.asarray(f, dtype=np.float32)
    x = np.asarray(x, dtype=np.float32)
    assert f.shape == (T, B, D) and x.shape == (T, B, D)

    nc = _get_nc()
    f_shards = _shard(f)
    x_shards = _shard(x)
    in_maps = [{"f_in": f_shards[c], "x_in": x_shards[c]} for c in range(NCORES)]
    res = run_bass_kernel_spmd(nc, in_maps, core_ids=list(range(NCORES)), trace=trace)

    out = np.empty((T, B, D), dtype=np.float32)
    for c in range(NCORES):
        # h_c[d, b, t_rev] -> out[t, b, DS*c + d]
        out[:, :, DS * c : DS * (c + 1)] = res.results[c]["h_out"][:, :, ::-1].transpose(2, 1, 0)
    return out.reshape(T * B, D), res


def kernel(f, x):
    return _run(f, x, trace=False)[0]


# revision 13
# speedup vs baseline: 1.2232x; 1.0730x over previous
"""Reverse-time forget-mult recurrence on 8 Trainium2 NeuronCores.

h_t = f_t*x_t + (1-f_t)*h_{t+1}, h_{T+1}=0, over [T=2048, B=16, D=1024].

Strategy: shard D across the 8 cores (128 channels each) — the recurrence is
elementwise over (B, D), sequential only in T, so no cross-core communication.
On the host, each core's shard is laid out partition-major as [D_shard=128,
B=16, T] with the T axis reversed, so each (d, b) lane's full time series is
contiguous and the device scans forward. Per 2-block step the device does one
contiguous 2 MB DMA per tensor (16 KB per-partition lines), computes
a = 1-f on the Scalar engine and g = f*x on the Vector engine, and runs the
whole recurrence for 128 lanes x 2048 steps in a single hardware
tensor_tensor_scan instruction (initial state 0) on Vector. Loads issue on
the Sync HWDGE ring, stores on the Scalar ring, so writes don't
head-of-line-block reads. The very last block is scanned/stored in chained
quarter-T chunks to shorten the pipeline drain. The kernel is memory-bound:
48 MB of HBM traffic per core.
"""

import numpy as np

T, B, D = 2048, 16, 1024
NCORES = 8
DS = D // NCORES          # 128 channels per core -> the SBUF partition dim
NBLK = B                  # 16 blocks of [128, T] per core
RB = 2                    # row-blocks per DMA (2 MB transfers)
PB = 128

_cached = {}


def _build():
    import concourse.bacc as bacc
    import concourse.mybir as mybir
    import concourse.tile as tile

    f32 = mybir.dt.float32
    nc = bacc.Bacc("TRN2", target_bir_lowering=False, debug=False, num_devices=NCORES)
    f_in = nc.dram_tensor("f_in", [PB, NBLK, T], f32, kind="ExternalInput").ap()
    x_in = nc.dram_tensor("x_in", [PB, NBLK, T], f32, kind="ExternalInput").ap()
    h_out = nc.dram_tensor("h_out", [PB, NBLK, T], f32, kind="ExternalOutput").ap()

    nsteps = NBLK // RB
    Q = T // 4
    with tile.TileContext(nc) as tc:
        with (
            tc.tile_pool(name="io", bufs=3) as io_pool,
            tc.tile_pool(name="hp", bufs=4) as h_pool,
            tc.tile_pool(name="tmp", bufs=3) as tmp_pool,
        ):
            for r in range(nsteps):
                bsl = slice(RB * r, RB * (r + 1))
                f_t = io_pool.tile([PB, RB, T], f32, tag="f")
                nc.sync.dma_start(out=f_t[:], in_=f_in[:, bsl, :])
                x_t = io_pool.tile([PB, RB, T], f32, tag="x")
                nc.sync.dma_start(out=x_t[:], in_=x_in[:, bsl, :])
                for j in range(RB):
                    blk = RB * r + j
                    a_t = tmp_pool.tile([PB, T], f32, tag="a")
                    nc.scalar.activation(
                        a_t[:], f_t[:, j, :],
                        mybir.ActivationFunctionType.Copy, bias=1.0, scale=-1.0,
                    )
                    g_t = tmp_pool.tile([PB, T], f32, tag="g")
                    nc.vector.tensor_mul(g_t[:], f_t[:, j, :], x_t[:, j, :])
                    h_t = h_pool.tile([PB, T], f32, tag="h")
                    if blk < NBLK - 1:
                        nc.vector.tensor_tensor_scan(
                            h_t[:], a_t[:], g_t[:], 0.0,
                            mybir.AluOpType.mult, mybir.AluOpType.add,
                        )
                        nc.scalar.dma_start(out=h_out[:, blk, :], in_=h_t[:])
                    else:
                        # last block: chained quarter-scans + quarter-stores
                        # to shorten the pipeline drain
                        for q in range(4):
                            qsl = slice(Q * q, Q * (q + 1))
                            init = 0.0 if q == 0 else h_t[:, Q * q - 1 : Q * q]
                            nc.vector.tensor_tensor_scan(
                                h_t[:, qsl], a_t[:, qsl], g_t[:, qsl], init,
                                mybir.AluOpType.mult, mybir.AluOpType.add,
                            )
                            nc.scalar.dma_start(
                                out=h_out[:, blk, qsl], in_=h_t[:, qsl]
                            )
    nc.compile()
    return nc


def _get_nc():
    if "nc" not in _cached:
        _cached["nc"] = _build()
    return _cached["nc"]


def _shard(arr):
    """[T, B, D] -> per-core [DS, B, T] (partition-major) with T reversed."""
    v = arr[::-1].transpose(2, 1, 0)  # [D, B, T] strided view, T reversed
    return [
        np.ascontiguousarray(v[DS * c : DS * (c + 1)]) for c in range(NCORES)
    ]


def _run(f, x, trace=False):
    from concourse.bass_utils import run_bass_kernel_spmd

    f = np.asarray(f, dtype=np.float32)
    x = np.asarray(x, dtype=np.float32)
    assert f.shape == (T, B, D) and x.shape == (T, B, D)

    nc = _get_nc()
    f_shards = _shard(f)
    x_shards = _shard(x)
    in_maps = [{"f_in": f_shards[c], "x_in": x_shards[c]} for c in range(NCORES)]
    res = run_bass_kernel_spmd(nc, in_maps, core_ids=list(range(NCORES)), trace=trace)

    out = np.empty((T, B, D), dtype=np.float32)
    for c in range(NCORES):
        # h_c[d, b, t_rev] -> out[t, b, DS*c + d]
        out[:, :, DS * c : DS * (c + 1)] = res.results[c]["h_out"][:, :, ::-1].transpose(2, 1, 0)
    return out.reshape(T * B, D), res


def kernel(f, x):
    return _run(f, x, trace=False)[0]


# revision 14
# speedup vs baseline: 1.2736x; 1.0412x over previous
"""Reverse-time forget-mult recurrence on 8 Trainium2 NeuronCores.

h_t = f_t*x_t + (1-f_t)*h_{t+1}, h_{T+1}=0, over [T=2048, B=16, D=1024].

Strategy: shard D across the 8 cores (128 channels each) — the recurrence is
elementwise over (B, D), sequential only in T, so no cross-core communication.
On the host, each core's shard is laid out partition-major as [D_shard=128,
B=16, T] with the T axis reversed, so each (d, b) lane's full time series is
contiguous and the device scans forward. Per 2-block step the device does one
contiguous 2 MB DMA per tensor (16 KB per-partition lines), computes
a = 1-f on the Scalar engine and g = f*x on the Vector engine, and runs the
whole recurrence for 128 lanes x 2048 steps in a single hardware
tensor_tensor_scan instruction (initial state 0) on Vector. Loads issue on
the Sync HWDGE ring, stores on the Scalar ring, so writes don't
head-of-line-block reads. The very last block is scanned/stored in chained
quarter-T chunks to shorten the pipeline drain, and the first two blocks'
stores are deferred to the kernel tail on the then-idle Sync ring, filling
the end-of-stream DMA gap while the final scans run. The kernel is
memory-bound: 48 MB of HBM traffic per core.
"""

import numpy as np

T, B, D = 2048, 16, 1024
NCORES = 8
DS = D // NCORES          # 128 channels per core -> the SBUF partition dim
NBLK = B                  # 16 blocks of [128, T] per core
RB = 2                    # row-blocks per DMA (2 MB transfers)
PB = 128

_cached = {}


def _build():
    import concourse.bacc as bacc
    import concourse.mybir as mybir
    import concourse.tile as tile

    f32 = mybir.dt.float32
    nc = bacc.Bacc("TRN2", target_bir_lowering=False, debug=False, num_devices=NCORES)
    f_in = nc.dram_tensor("f_in", [PB, NBLK, T], f32, kind="ExternalInput").ap()
    x_in = nc.dram_tensor("x_in", [PB, NBLK, T], f32, kind="ExternalInput").ap()
    h_out = nc.dram_tensor("h_out", [PB, NBLK, T], f32, kind="ExternalOutput").ap()

    nsteps = NBLK // RB
    Q = T // 4
    with tile.TileContext(nc) as tc:
        with (
            tc.tile_pool(name="io", bufs=3) as io_pool,
            tc.tile_pool(name="hp", bufs=4) as h_pool,
            tc.tile_pool(name="hd", bufs=1) as hd_pool,
            tc.tile_pool(name="tmp", bufs=3) as tmp_pool,
        ):
            deferred = {}
            for r in range(nsteps):
                bsl = slice(RB * r, RB * (r + 1))
                f_t = io_pool.tile([PB, RB, T], f32, tag="f")
                nc.sync.dma_start(out=f_t[:], in_=f_in[:, bsl, :])
                x_t = io_pool.tile([PB, RB, T], f32, tag="x")
                nc.sync.dma_start(out=x_t[:], in_=x_in[:, bsl, :])
                if r == nsteps - 1:
                    # the Sync ring is idle after the final load: flush the
                    # deferred block-0 store there to fill the end DMA gap
                    for dblk, dh in deferred.items():
                        nc.sync.dma_start(out=h_out[:, dblk, :], in_=dh[:])
                for j in range(RB):
                    blk = RB * r + j
                    a_t = tmp_pool.tile([PB, T], f32, tag="a", bufs=2)
                    nc.scalar.activation(
                        a_t[:], f_t[:, j, :],
                        mybir.ActivationFunctionType.Copy, bias=1.0, scale=-1.0,
                    )
                    g_t = tmp_pool.tile([PB, T], f32, tag="g")
                    nc.vector.tensor_mul(g_t[:], f_t[:, j, :], x_t[:, j, :])
                    if blk <= 1:
                        h_t = hd_pool.tile([PB, T], f32, tag=f"hd{blk}", name=f"hd{blk}")
                    else:
                        h_t = h_pool.tile([PB, T], f32, tag="h")
                    if blk < NBLK - 1:
                        nc.vector.tensor_tensor_scan(
                            h_t[:], a_t[:], g_t[:], 0.0,
                            mybir.AluOpType.mult, mybir.AluOpType.add,
                        )
                        if blk <= 1:
                            deferred[blk] = h_t
                        else:
                            nc.scalar.dma_start(out=h_out[:, blk, :], in_=h_t[:])
                    else:
                        # last block: chained quarter-scans + quarter-stores
                        # to shorten the pipeline drain
                        for q in range(4):
                            qsl = slice(Q * q, Q * (q + 1))
                            init = 0.0 if q == 0 else h_t[:, Q * q - 1 : Q * q]
                            nc.vector.tensor_tensor_scan(
                                h_t[:, qsl], a_t[:, qsl], g_t[:, qsl], init,
                                mybir.AluOpType.mult, mybir.AluOpType.add,
                            )
                            nc.scalar.dma_start(
                                out=h_out[:, blk, qsl], in_=h_t[:, qsl]
                            )
    nc.compile()
    return nc


def _get_nc():
    if "nc" not in _cached:
        _cached["nc"] = _build()
    return _cached["nc"]


def _shard(arr):
    """[T, B, D] -> per-core [DS, B, T] (partition-major) with T reversed."""
    v = arr[::-1].transpose(2, 1, 0)  # [D, B, T] strided view, T reversed
    return [
        np.ascontiguousarray(v[DS * c : DS * (c + 1)]) for c in range(NCORES)
    ]


def _run(f, x, trace=False):
    from concourse.bass_utils import run_bass_kernel_spmd

    f = np.asarray(f, dtype=np.float32)
    x = np.asarray(x, dtype=np.float32)
    assert f.shape == (T, B, D) and x.shape == (T, B, D)

    nc = _get_nc()
    f_shards = _shard(f)
    x_shards = _shard(x)
    in_maps = [{"f_in": f_shards[c], "x_in": x_shards[c]} for c in range(NCORES)]
    res = run_bass_kernel_spmd(nc, in_maps, core_ids=list(range(NCORES)), trace=trace)

    out = np.empty((T, B, D), dtype=np.float32)
    for c in range(NCORES):
        # h_c[d, b, t_rev] -> out[t, b, DS*c + d]
        out[:, :, DS * c : DS * (c + 1)] = res.results[c]["h_out"][:, :, ::-1].transpose(2, 1, 0)
    return out.reshape(T * B, D), res


def kernel(f, x):
    return _run(f, x, trace=False)[0]
